# revision 38
# baseline (speedup 1.0000x reference)
"""Causal self-attention with RoPE — Trainium2 Bass/Tile kernel.

Problem: B=2, T=2048, C=2048, H=16 heads, D=128 head dim.
    qkv = x @ w_qkv ; RoPE(q, k) ; causal softmax attention ; out = attn_out @ w_out

Sharding (8 cores): core c handles batch b = c//4 and the 4 heads
hg = c%4 (heads 4*hg .. 4*hg+3).  Each core computes
    partial_c = attn_bh(x[b]) @ w_out[rows of its heads]      (shape [T, C])
and the host all-reduces: out[b] = sum of the 4 partials of batch b.

Per-core pipeline (all matmuls bf16/fp16 inputs, fp32 PSUM accumulate):
  A) QKV projection.  q,k produced transposed ([D, T], feature-major) so that
     scores/attn matmuls need no transposes; v produced natural ([T, D]).
  B) RoPE applied in [D, T] layout (partition-shift + cos/sin tables).
  C) Flash-style causal attention per (head, 512-query chunk), qc-OUTER:
     scoresT[k,q] blocks via matmul, exp on ScalarE (PSUM->SBUF, bf16,
     bias=-4 to keep fp16 row sums small), tri-mask on diagonal blocks (DVE),
     attn@v accumulated in PSUM.  Softmax denominators: DVE accumulates the
     exp tiles into an fp16 per-partition partial sum (4x DVE mode), then ONE
     all-ones [128,128] fp16 matmul broadcasts the cross-partition total
     (PE cost 1/10th of the old two-ones-matmuls-per-pair scheme).
     reciprocal_approx_fast (DVE, ~5x faster than reciprocal) + multiply
     -> normalized outT (bf16).
  D) Out-projection -> partial [T, C] fp32.  Emitted INTERLEAVED into the
     NEXT query chunk's attention pair loop so its matmuls fill the PE
     bubbles where PE would wait on ScalarE exps; PSUM->SBUF copies run on
     the otherwise-idle Pool engine; DMA out on the SP ring.
"""

import sys

for _p in ("/opt/trn_rl_repo",):
    if _p not in sys.path:
        sys.path.insert(0, _p)

import numpy as np
import ml_dtypes

import concourse.bass as bass
import concourse.mybir as mybir
import concourse.tile as tile

BF = mybir.dt.bfloat16
F16 = mybir.dt.float16
FP = mybir.dt.float32

BF_NP = ml_dtypes.bfloat16
F16_NP = np.float16

NUM_HEADS = 16
B, T_FULL, C_FULL = 2, 2048, 2048
D = 128
N_CORES = 8
HPC = 4  # heads per core

ROPE_THETA = 10000.0
EXP_BIAS = -4.0  # exp(s*scale - 4): cancels in softmax, keeps fp16 sums small


def _split_multi_waits(nc):
    """This container's walrus supports only ONE sync-wait per instruction
    ("Too many sync wait commands").  Hoist all but one wait of every
    multi-wait instruction onto preceding EventSemaphore instructions
    executed by the same engine's sequencer (block order = program order per
    engine) — same semantics, codegen-legal."""
    import bass_rust

    skip = (mybir.InstEventSemaphore,)
    ctr = 0
    for fn in nc.m.functions:
        for blk in fn.blocks:
            new_insts = None
            for idx, inst in enumerate(blk.instructions):
                si = inst.sync_info
                if (
                    not isinstance(inst, skip)
                    and si is not None
                    and si.on_wait
                    and len(si.on_wait) > 1
                ):
                    if new_insts is None:
                        new_insts = list(blk.instructions[:idx])
                    # keep the first wait (the data-dep one, usually latest to
                    # resolve) on the instruction itself; hoist the rest.
                    for w in si.on_wait[1:]:
                        ev = mybir.InstEventSemaphore(
                            name=f"I-dmaw{ctr}", ins=[], outs=[]
                        )
                        ctr += 1
                        ev.sync_info = bass_rust.SyncInfo(
                            on_wait=[w], on_update=[]
                        )
                        ev.engine = inst.engine
                        new_insts.append(ev)
                    inst.sync_info = bass_rust.SyncInfo(
                        on_wait=[si.on_wait[0]], on_update=si.on_update or []
                    )
                    new_insts.append(inst)
                elif new_insts is not None:
                    new_insts.append(inst)
            if new_insts is not None:
                blk.instructions = new_insts


class Cfg:
    """Kernel geometry. Full-size by default; shrinkable for simulator tests."""

    def __init__(self, T=T_FULL, C=C_FULL, hpc=HPC):
        assert T % 512 == 0 and C % 128 == 0
        self.T = T
        self.C = C
        self.hpc = hpc
        self.scale = 1.0 / np.sqrt(D)
        self.c_tiles = C // 128      # contraction tiles for QKV
        self.t_chunks = T // 512     # token chunks (QKV + queries)
        self.t_tiles = T // 128      # token tiles (keys / out rows)
        self.n_chunks = C // 512     # output-feature chunks for out-proj


def build_attention(cfg: Cfg):
    """Build the SPMD Bass program (identical on all cores; data differs)."""
    nc = bass.Bass("TRN2", debug=False, enable_partition_id=False)
    T, C, hpc = cfg.T, cfg.C, cfg.hpc
    F = hpc * D  # per-core q (or k, or v) feature count

    xT = nc.dram_tensor("xT", [C // 128, 128, T], BF, kind="ExternalInput")
    # wqk pre-packed per output-feature tile: [ft, p, (cc f)] so one 2D DMA
    # fetches one ft's full [C-chunk=128, C] weight tile.
    wqk = nc.dram_tensor("wqk", [2 * hpc * 128, C], BF, kind="ExternalInput")
    wv = nc.dram_tensor("wv", [C, F], BF, kind="ExternalInput")
    wout = nc.dram_tensor("wout", [F, C], BF, kind="ExternalInput")
    cosT = nc.dram_tensor("cosT", [D, T], BF, kind="ExternalInput")
    sinT = nc.dram_tensor("sinT", [D, T], BF, kind="ExternalInput")  # sign-baked
    masks = nc.dram_tensor("masks", [128, 128], F16, kind="ExternalInput")
    ones = nc.dram_tensor("ones", [128, 128], F16, kind="ExternalInput")
    out = nc.dram_tensor("out", [T, C], FP, kind="ExternalOutput")

    Exp = mybir.ActivationFunctionType.Exp
    Ln = mybir.ActivationFunctionType.Ln

    with tile.TileContext(nc) as tc:
        with (
            tc.tile_pool(name="consts", bufs=1) as consts,
            tc.tile_pool(name="persist", bufs=1) as persist,
            tc.tile_pool(name="otp", bufs=1) as otp,
            tc.tile_pool(name="wo_pool", bufs=1) as wo_pool,
            tc.tile_pool(name="wqk_pool", bufs=1) as wqk_pool,
            tc.tile_pool(name="wv_pool", bufs=1) as wv_pool,
        ):
            # weights on the ACT hwdge ring (ordered first: needed first);
            # x / outputs on the SP ring.
            wqkf_sb = [
                wqk_pool.tile([128, C], BF, name=f"wqkf_sb{ft}", tag=f"wqk{ft}")
                for ft in range(2 * hpc)
            ]
            for ft in range(2 * hpc):
                nc.scalar.dma_start(
                    out=wqkf_sb[ft], in_=wqk[ft * 128 : (ft + 1) * 128, :]
                )
            wv_sb = [
                wv_pool.tile([128, F], BF, name=f"wv_sb{cc}", tag=f"wv{cc}")
                for cc in range(cfg.c_tiles)
            ]
            for cc in range(cfg.c_tiles):
                nc.scalar.dma_start(
                    out=wv_sb[cc], in_=wv[cc * 128 : (cc + 1) * 128, :]
                )
            cos_sb = consts.tile([D, T], BF, name="cos_sb")
            nc.scalar.dma_start(out=cos_sb, in_=cosT[:, :])
            sin_sb = consts.tile([D, T], BF, name="sin_sb")
            nc.scalar.dma_start(out=sin_sb, in_=sinT[:, :])
            wout_sb = [
                wo_pool.tile([128, C], BF, name=f"wout_sb{h}", tag=f"wo{h}")
                for h in range(hpc)
            ]
            for h in range(hpc):
                nc.scalar.dma_start(
                    out=wout_sb[h], in_=wout[h * 128 : (h + 1) * 128, :]
                )
            tri_sb = consts.tile([128, 128], F16, name="tri_sb")
            nc.scalar.dma_start(out=tri_sb, in_=masks[:, :])
            ones_sb = consts.tile([128, 128], F16, name="ones_sb")
            nc.scalar.dma_start(out=ones_sb, in_=ones[:, :])
            ebias_sb = consts.tile([128, 1], FP, name="ebias_sb")
            nc.gpsimd.memset(ebias_sb, EXP_BIAS)

            # q/k transposed [D, T] per head (RoPE applied in place later);
            # v natural [T, F] stored as [128, t_tiles, F].
            qk_t = [
                persist.tile([D, T], BF, name=f"qk_t{ft}", tag=f"qk_t{ft}")
                for ft in range(2 * hpc)
            ]
            # v and the exp tiles are fp16 (not bf16): uniform 2-byte dtype
            # across DVE operands enables the DVE 2x/4x fast modes
            v_sb = persist.tile([128, cfg.t_tiles, F], F16, name="v_sb")

            def rope_inplace(ft, rope_tmp):
                t_cos = rope_tmp.tile([D, T], BF, tag="t_cos")
                nc.vector.tensor_mul(t_cos, qk_t[ft], cos_sb)
                t_shift = rope_tmp.tile([D, T], BF, tag="t_shift")
                nc.vector.tensor_copy(t_shift[0:64, :], qk_t[ft][64:128, :])
                nc.vector.tensor_copy(t_shift[64:128, :], qk_t[ft][0:64, :])
                nc.vector.tensor_mul(t_shift, t_shift, sin_sb)
                nc.vector.tensor_add(qk_t[ft], t_cos, t_shift)

            # ---------------- Phase A: QKV projection + RoPE ----------------
            with (
                tc.tile_pool(name="xpool", bufs=3) as xpool,
                tc.tile_pool(name="rope_tmp", bufs=2) as rope_tmp,
            ):

                def load_x_chunk(tci):
                    x_ch = []
                    for cc in range(cfg.c_tiles):
                        x_t = xpool.tile([128, 512], BF, tag=f"x{cc}")
                        nc.sync.dma_start(
                            out=x_t,
                            in_=xT[cc, :, tci * 512 : (tci + 1) * 512],
                        )
                        x_ch.append(x_t)
                    return x_ch

                # A1 chunk 0, cc-OUTER with one PSUM bank per output row: PE
                # consumes each x tile fully as it lands (~1.8us/tile) instead
                # of sweeping all 16 at DMA-issue rate 8 times, which stalled
                # the warm-up ~4.5us
                with tc.tile_pool(name="psA", bufs=1, space="PSUM") as psA:
                    x_ch = load_x_chunk(0)
                    ps_ft = [
                        psA.tile([128, 512], FP, name=f"psft{ft}", tag=f"psft{ft}")
                        for ft in range(2 * hpc)
                    ]
                    for cc in range(cfg.c_tiles):
                        for ft in range(2 * hpc):
                            nc.tensor.matmul(
                                ps_ft[ft],
                                lhsT=wqkf_sb[ft][:, cc * 128 : (cc + 1) * 128],
                                rhs=x_ch[cc],
                                start=(cc == 0),
                                stop=(cc == cfg.c_tiles - 1),
                            )
                    for ft in range(2 * hpc):
                        nc.vector.tensor_copy(qk_t[ft][:, 0:512], ps_ft[ft])
                        if cfg.t_chunks == 1:
                            rope_inplace(ft, rope_tmp)

                with tc.tile_pool(name="qkv_ps", bufs=3, space="PSUM") as qkv_ps:
                    # A1 remaining chunks: ft-outer, RoPE once each row done
                    for tci in range(1, cfg.t_chunks):
                        x_ch = load_x_chunk(tci)
                        for ft in range(2 * hpc):
                            ps_qk = qkv_ps.tile([128, 512], FP, tag="ps_qk")
                            for cc in range(cfg.c_tiles):
                                nc.tensor.matmul(
                                    ps_qk,
                                    lhsT=wqkf_sb[ft][:, cc * 128 : (cc + 1) * 128],
                                    rhs=x_ch[cc],
                                    start=(cc == 0),
                                    stop=(cc == cfg.c_tiles - 1),
                                )
                            nc.vector.tensor_copy(
                                qk_t[ft][:, tci * 512 : (tci + 1) * 512], ps_qk
                            )
                            if tci == cfg.t_chunks - 1:
                                rope_inplace(ft, rope_tmp)

                    # A2: v (natural); x re-streamed; ropes overlap on DVE
                    for tci in range(cfg.t_chunks):
                        x_ch = load_x_chunk(tci)
                        for tt in range(4):
                            ps_v = qkv_ps.tile([128, F], FP, tag="ps_v")
                            for cc in range(cfg.c_tiles):
                                nc.tensor.matmul(
                                    ps_v,
                                    lhsT=x_ch[cc][:, tt * 128 : (tt + 1) * 128],
                                    rhs=wv_sb[cc],
                                    start=(cc == 0),
                                    stop=(cc == cfg.c_tiles - 1),
                                )
                            nc.scalar.copy(v_sb[:, tci * 4 + tt, :], ps_v)

            # ---------- Phase B+D: causal attention w/ interleaved out-proj ----------
            # scoresT blocks [k, q] so attn@v needs no transposes; exp runs
            # 1024-wide over PAIRS of 128-key tiles; DVE accumulates fp16
            # partial row-sums; one fp16 ones-matmul broadcasts the total.
            otn = [[None] * cfg.t_chunks for _ in range(hpc)]
            with (
                tc.tile_pool(name="expp", bufs=6) as expp,
                tc.tile_pool(name="accp", bufs=2) as accp,
                tc.tile_pool(name="rsp", bufs=2) as rsp,
                tc.tile_pool(name="osb_pool", bufs=2) as osb_pool,
                tc.tile_pool(name="sc_ps", bufs=2, space="PSUM") as sc_ps,
                tc.tile_pool(name="av_ps", bufs=2, space="PSUM") as av_ps,
                tc.tile_pool(name="mx_ps", bufs=2, space="PSUM") as mx_ps,
            ):

                def outproj_emitters(qc):
                    """Closures each emitting one PE-chunk of out-proj for
                    query chunk qc (4 accum matmuls + Pool copy; last group
                    of each token tile also emits the DMA out)."""
                    ems = []
                    for tt_local in range(4):
                        tt = qc * 4 + tt_local
                        off = tt_local * 128
                        holder = {}
                        for n in range(cfg.n_chunks):
                            def group(holder=holder, qc=qc, off=off, n=n, tt=tt):
                                if n == 0:
                                    holder["sb"] = osb_pool.tile(
                                        [128, C], FP, name="out_sb", tag="out_sb"
                                    )
                                ps_o = mx_ps.tile(
                                    [128, 512], FP, name="ps_o", tag="mx"
                                )
                                for h in range(hpc):
                                    nc.tensor.matmul(
                                        ps_o,
                                        lhsT=otn[h][qc][:, off : off + 128],
                                        rhs=wout_sb[h][:, n * 512 : (n + 1) * 512],
                                        start=(h == 0),
                                        stop=(h == hpc - 1),
                                    )
                                # PSUM->SBUF copies split 1:3 Act:DVE so
                                # neither engine becomes the bottleneck
                                dst = holder["sb"][:, n * 512 : (n + 1) * 512]
                                if n == 0:
                                    nc.scalar.copy(dst, ps_o)
                                else:
                                    nc.vector.tensor_copy(dst, ps_o)
                                # stream the result out in halves so the final
                                # DMA tail is short
                                half = C // 2 if cfg.n_chunks >= 2 else 0
                                if cfg.n_chunks >= 2 and 2 * (n + 1) == cfg.n_chunks:
                                    nc.sync.dma_start(
                                        out=out[tt * 128 : (tt + 1) * 128, 0:half],
                                        in_=holder["sb"][:, 0:half],
                                    )
                                elif n == cfg.n_chunks - 1:
                                    nc.sync.dma_start(
                                        out=out[tt * 128 : (tt + 1) * 128, half:C],
                                        in_=holder["sb"][:, half:C],
                                    )
                            ems.append(group)
                    return ems

                pending = []  # out-proj emitters for the previous query chunk
                pending_fin = [None]  # lazy per-head softmax finisher

                def make_finisher(h, qc, ps_av, acc):
                    def fin():
                        # cross-partition rowsum total via ONE all-ones fp16
                        # matmul; 1/x = exp(-ln(x)) on ScalarE (the custom-DVE
                        # fast reciprocal doesn't compile on this walrus and
                        # the native DVE reciprocal costs 3.4us)
                        ps_rs = mx_ps.tile(
                            [128, 512], FP, name="ps_rs", tag="mx"
                        )
                        nc.tensor.matmul(
                            ps_rs, lhsT=ones_sb, rhs=acc, start=True, stop=True
                        )
                        rsln = rsp.tile([128, 512], FP, name="rsln", tag="rsln")
                        nc.scalar.activation(rsln, ps_rs, Ln)
                        rsrec = rsp.tile([128, 512], FP, name="rsrec", tag="rsrec")
                        nc.scalar.activation(rsrec, rsln, Exp, scale=-1.0)
                        o = otp.tile(
                            [128, 512], BF, name=f"otn{h}_{qc}", tag=f"otn{h}_{qc}"
                        )
                        nc.vector.tensor_mul(o, ps_av, rsrec)
                        otn[h][qc] = o
                    return fin

                for qc in range(cfg.t_chunks):
                    nkp = (qc + 1) * 2  # causal: pairs of key tiles
                    n_pairs = hpc * nkp
                    pi = 0  # pair counter within this qc
                    emitted = 0
                    # per-tile (key-tile j, q-valid-start o, width w): the 4
                    # diagonal tiles only need queries >= their own offset
                    tiles = [
                        (j, max(0, j * 128 - qc * 512))
                        for j in range(2 * nkp)
                    ]
                    for h in range(hpc):
                        q_h = qk_t[h]
                        k_h = qk_t[hpc + h]
                        q_sl = q_h[:, qc * 512 : (qc + 1) * 512]
                        ps_av = av_ps.tile([128, 512], FP, tag="ps_av")
                        acc = accp.tile([128, 512], F16, tag="acc")

                        def emit_av_adds(jp, expT, ps_av=ps_av, acc=acc, h=h):
                            pr = [tiles[2 * jp], tiles[2 * jp + 1]]
                            w0 = 512 - pr[0][1]
                            first, last = (jp == 0), (jp == nkp - 1)
                            for ti, (base, (j, o)) in enumerate(
                                ((0, pr[0]), (w0, pr[1]))
                            ):
                                nc.tensor.matmul(
                                    ps_av[:, o:512],
                                    lhsT=v_sb[:, j, h * 128 : (h + 1) * 128],
                                    rhs=expT[:, base : base + 512 - o],
                                    start=(first and ti == 0),
                                    stop=(last and ti == 1),
                                )
                            # fp16 partial row-sums on DVE (fast-mode, SBUF)
                            for ti, (base, (j, o)) in enumerate(
                                ((0, pr[0]), (w0, pr[1]))
                            ):
                                if first and ti == 0:
                                    nc.vector.tensor_copy(acc, expT[:, 0:512])
                                else:
                                    nc.vector.tensor_add(
                                        acc[:, o:512],
                                        acc[:, o:512],
                                        expT[:, base : base + 512 - o],
                                    )

                        la = []  # (jp, expT) whose attn@v emission is deferred
                        for jp in range(nkp):
                            pr = [tiles[2 * jp], tiles[2 * jp + 1]]
                            w0, w1 = (512 - pr[0][1]), (512 - pr[1][1])
                            ew = w0 + w1
                            ps_sc = sc_ps.tile([128, 1024], FP, tag="ps_sc")
                            for base, (j, o) in ((0, pr[0]), (w0, pr[1])):
                                nc.tensor.matmul(
                                    ps_sc[:, base : base + 512 - o],
                                    lhsT=k_h[:, j * 128 : (j + 1) * 128],
                                    rhs=q_sl[:, o:512],
                                    start=True,
                                    stop=True,
                                )
                            expT = expp.tile([128, 1024], F16, tag="expT")
                            nc.scalar.activation(
                                expT[:, 0:ew], ps_sc[:, 0:ew], Exp,
                                scale=float(cfg.scale), bias=ebias_sb,
                            )
                            # triangular mask on each diagonal tile's leading
                            # 128 queries
                            for base, (j, o) in ((0, pr[0]), (w0, pr[1])):
                                if o or (j * 128 == qc * 512):
                                    nc.vector.tensor_mul(
                                        expT[:, base : base + 128],
                                        expT[:, base : base + 128],
                                        tri_sb,
                                    )
                            # one-pair lookahead: defer attn@v emission so PE
                            # never catches up with ScalarE's exps
                            la.append((jp, expT))
                            if len(la) > 1:
                                emit_av_adds(*la.pop(0))
                            # finish the PREVIOUS head's softmax one pair into
                            # this head (keeps its Act/DVE tail off the
                            # critical path between heads)
                            if jp == 0 and pending_fin[0] is not None:
                                pending_fin[0]()
                                pending_fin[0] = None
                            # interleave out-proj of the previous chunk into
                            # the PE stream to fill exp-wait bubbles
                            pi += 1
                            while pending and emitted < len(pending) * pi // n_pairs:
                                pending[emitted]()
                                emitted += 1
                        for item in la:
                            emit_av_adds(*item)
                        la = []
                        pending_fin[0] = make_finisher(h, qc, ps_av, acc)
                    for em in pending[emitted:]:
                        em()
                    pending = outproj_emitters(qc)
                # flush the last head's finisher, then the last out-proj
                pending_fin[0]()
                pending_fin[0] = None
                for em in pending:
                    em()

    return nc


def rope_tables(T, dtype=np.float32):
    inv_freq = 1.0 / (ROPE_THETA ** (np.arange(0, D, 2, dtype=np.float32) / D))
    t = np.arange(T, dtype=np.float32)
    freqs = np.outer(t, inv_freq)  # [T, D/2]
    emb = np.concatenate([freqs, freqs], axis=-1)  # [T, D]
    return np.cos(emb).astype(dtype), np.sin(emb).astype(dtype)


def make_core_inputs(cfg: Cfg, x_b, w_qkv, w_out, cos, sin, hg):
    """Per-core input dict. x_b [T, C] fp32; w_qkv [C, 3C']; w_out [C', C];
    cos/sin [T, D]; hg = head-group index within the batch group."""
    T, C, hpc = cfg.T, cfg.C, cfg.hpc
    F = hpc * D
    H = w_qkv.shape[1] // 3 // D  # total heads in this (possibly shrunk) problem
    CQ = H * D

    f0 = hg * F
    xT = np.ascontiguousarray(x_b.T).astype(BF_NP).reshape(C // 128, 128, T)
    wq = w_qkv[:, f0 : f0 + F]
    wk = w_qkv[:, CQ + f0 : CQ + f0 + F]
    W = np.concatenate([wq, wk], axis=1)  # [C, 2F]
    # pack per-ft: wqk[ft*128+p, cc*128+f] = W[cc*128+p, ft*128+f]
    nft, ncc = 2 * hpc, C // 128
    wqk = np.ascontiguousarray(
        W.reshape(ncc, 128, nft, 128).transpose(2, 1, 0, 3).reshape(
            nft * 128, ncc * 128
        )
    ).astype(BF_NP)
    wv = np.ascontiguousarray(w_qkv[:, 2 * CQ + f0 : 2 * CQ + f0 + F]).astype(BF_NP)
    wout = np.ascontiguousarray(w_out[f0 : f0 + F, :]).astype(BF_NP)

    cosT = np.ascontiguousarray(cos.T).astype(BF_NP)  # [D, T]
    sinT = np.ascontiguousarray(sin.T).astype(np.float32)
    sinT[0:64, :] *= -1.0  # bake rotate_half sign
    sinT = sinT.astype(BF_NP)

    # triangular mask for diagonal 128x128 blocks: tri[k, q] = 1 iff k <= q
    k_idx = np.arange(128)[:, None]
    q_idx = np.arange(128)[None, :]
    m = (k_idx <= q_idx).astype(F16_NP)

    return {
        "xT": xT,
        "wqk": wqk,
        "wv": wv,
        "wout": wout,
        "cosT": cosT,
        "sinT": sinT,
        "masks": np.ascontiguousarray(m),
        "ones": np.ones((128, 128), dtype=F16_NP),
    }


_NC_CACHE = {}


def _get_nc(cfg: Cfg):
    key = (cfg.T, cfg.C, cfg.hpc)
    if key not in _NC_CACHE:
        nc = build_attention(cfg)
        _split_multi_waits(nc)  # HW codegen needs ≤1 wait per instruction
        _NC_CACHE[key] = nc
    return _NC_CACHE[key]


def kernel(x, cos, sin, w_qkv, w_out, trace=False, tmpdir=None):
    """Full-problem entry point: full inputs in, full [B, T, C] output back."""
    from concourse.bass_utils import run_bass_kernel_spmd

    x = np.asarray(x, dtype=np.float32)
    cos = np.asarray(cos, dtype=np.float32)
    sin = np.asarray(sin, dtype=np.float32)
    w_qkv = np.asarray(w_qkv, dtype=np.float32)
    w_out = np.asarray(w_out, dtype=np.float32)

    cfg = Cfg()
    nc = _get_nc(cfg)

    in_maps = []
    for c in range(N_CORES):
        b, hg = c // 4, c % 4
        in_maps.append(
            make_core_inputs(cfg, x[b], w_qkv, w_out, cos, sin, hg)
        )

    res = run_bass_kernel_spmd(
        nc,
        in_maps,
        core_ids=list(range(N_CORES)),
        trace=trace,
        tmpdir=tmpdir,
    )
    partials = [r["out"] for r in res.results]
    out = np.empty((B, cfg.T, cfg.C), dtype=np.float32)
    for b in range(B):
        out[b] = partials[4 * b] + partials[4 * b + 1]
        out[b] += partials[4 * b + 2]
        out[b] += partials[4 * b + 3]
    if trace:
        return out, res
    return out


# revision 40
# speedup vs baseline: 1.0403x; 1.0403x over previous
"""Causal self-attention with RoPE — Trainium2 Bass/Tile kernel.

Problem: B=2, T=2048, C=2048, H=16 heads, D=128 head dim.
    qkv = x @ w_qkv ; RoPE(q, k) ; causal softmax attention ; out = attn_out @ w_out

Sharding (8 cores): core c handles batch b = c//4 and the 4 heads
hg = c%4 (heads 4*hg .. 4*hg+3).  Each core computes
    partial_c = attn_bh(x[b]) @ w_out[rows of its heads]      (shape [T, C])
and the host all-reduces: out[b] = sum of the 4 partials of batch b.

Per-core pipeline (all matmuls bf16/fp16 inputs, fp32 PSUM accumulate):
  A) QKV projection.  q,k produced transposed ([D, T], feature-major) so that
     scores/attn matmuls need no transposes; v produced natural ([T, D]).
  B) RoPE applied in [D, T] layout (partition-shift + cos/sin tables).
  C) Flash-style causal attention per (head, 512-query chunk), qc-OUTER:
     scoresT[k,q] blocks via matmul, exp on ScalarE (PSUM->SBUF, bf16,
     bias=-4 to keep fp16 row sums small), tri-mask on diagonal blocks (DVE),
     attn@v accumulated in PSUM.  Softmax denominators: DVE accumulates the
     exp tiles into an fp16 per-partition partial sum (4x DVE mode), then ONE
     all-ones [128,128] fp16 matmul broadcasts the cross-partition total
     (PE cost 1/10th of the old two-ones-matmuls-per-pair scheme).
     reciprocal_approx_fast (DVE, ~5x faster than reciprocal) + multiply
     -> normalized outT (bf16).
  D) Out-projection -> partial [T, C] fp32.  Emitted INTERLEAVED into the
     NEXT query chunk's attention pair loop so its matmuls fill the PE
     bubbles where PE would wait on ScalarE exps; PSUM->SBUF copies run on
     the otherwise-idle Pool engine; DMA out on the SP ring.
"""

import sys

for _p in ("/opt/trn_rl_repo",):
    if _p not in sys.path:
        sys.path.insert(0, _p)

import numpy as np
import ml_dtypes

import concourse.bass as bass
import concourse.mybir as mybir
import concourse.tile as tile

BF = mybir.dt.bfloat16
F16 = mybir.dt.float16
FP = mybir.dt.float32

BF_NP = ml_dtypes.bfloat16
F16_NP = np.float16

NUM_HEADS = 16
B, T_FULL, C_FULL = 2, 2048, 2048
D = 128
N_CORES = 8
HPC = 4  # heads per core

ROPE_THETA = 10000.0
EXP_BIAS = -4.0  # exp(s*scale - 4): cancels in softmax, keeps fp16 sums small


def _split_multi_waits(nc):
    """This container's walrus supports only ONE sync-wait per instruction
    ("Too many sync wait commands").  Hoist all but one wait of every
    multi-wait instruction onto preceding EventSemaphore instructions
    executed by the same engine's sequencer (block order = program order per
    engine) — same semantics, codegen-legal."""
    import bass_rust

    skip = (mybir.InstEventSemaphore,)
    ctr = 0
    for fn in nc.m.functions:
        for blk in fn.blocks:
            new_insts = None
            for idx, inst in enumerate(blk.instructions):
                si = inst.sync_info
                if (
                    not isinstance(inst, skip)
                    and si is not None
                    and si.on_wait
                    and len(si.on_wait) > 1
                ):
                    if new_insts is None:
                        new_insts = list(blk.instructions[:idx])
                    # keep the first wait (the data-dep one, usually latest to
                    # resolve) on the instruction itself; hoist the rest.
                    for w in si.on_wait[1:]:
                        ev = mybir.InstEventSemaphore(
                            name=f"I-dmaw{ctr}", ins=[], outs=[]
                        )
                        ctr += 1
                        ev.sync_info = bass_rust.SyncInfo(
                            on_wait=[w], on_update=[]
                        )
                        ev.engine = inst.engine
                        new_insts.append(ev)
                    inst.sync_info = bass_rust.SyncInfo(
                        on_wait=[si.on_wait[0]], on_update=si.on_update or []
                    )
                    new_insts.append(inst)
                elif new_insts is not None:
                    new_insts.append(inst)
            if new_insts is not None:
                blk.instructions = new_insts


class Cfg:
    """Kernel geometry. Full-size by default; shrinkable for simulator tests."""

    def __init__(self, T=T_FULL, C=C_FULL, hpc=HPC):
        assert T % 512 == 0 and C % 128 == 0
        self.T = T
        self.C = C
        self.hpc = hpc
        self.scale = 1.0 / np.sqrt(D)
        self.c_tiles = C // 128      # contraction tiles for QKV
        self.t_chunks = T // 512     # token chunks (QKV + queries)
        self.t_tiles = T // 128      # token tiles (keys / out rows)
        self.n_chunks = C // 512     # output-feature chunks for out-proj


def build_attention(cfg: Cfg):
    """Build the SPMD Bass program (identical on all cores; data differs)."""
    nc = bass.Bass("TRN2", debug=False, enable_partition_id=False)
    T, C, hpc = cfg.T, cfg.C, cfg.hpc
    F = hpc * D  # per-core q (or k, or v) feature count

    xT = nc.dram_tensor("xT", [C // 128, 128, T], BF, kind="ExternalInput")
    # wqk pre-packed per output-feature tile: [ft, p, (cc f)] so one 2D DMA
    # fetches one ft's full [C-chunk=128, C] weight tile.
    wqk = nc.dram_tensor("wqk", [2 * hpc * 128, C], BF, kind="ExternalInput")
    wv = nc.dram_tensor("wv", [C, F], BF, kind="ExternalInput")
    wout = nc.dram_tensor("wout", [F, C], BF, kind="ExternalInput")
    cosT = nc.dram_tensor("cosT", [D, T], BF, kind="ExternalInput")
    sinT = nc.dram_tensor("sinT", [D, T], BF, kind="ExternalInput")  # sign-baked
    masks = nc.dram_tensor("masks", [128, 128], F16, kind="ExternalInput")
    ones = nc.dram_tensor("ones", [128, 128], F16, kind="ExternalInput")
    out = nc.dram_tensor("out", [T, C], FP, kind="ExternalOutput")

    Exp = mybir.ActivationFunctionType.Exp
    Ln = mybir.ActivationFunctionType.Ln

    with tile.TileContext(nc) as tc:
        with (
            tc.tile_pool(name="consts", bufs=1) as consts,
            tc.tile_pool(name="persist", bufs=1) as persist,
            tc.tile_pool(name="otp", bufs=1) as otp,
            tc.tile_pool(name="wo_pool", bufs=1) as wo_pool,
            tc.tile_pool(name="wqk_pool", bufs=1) as wqk_pool,
            tc.tile_pool(name="wv_pool", bufs=1) as wv_pool,
        ):
            # weights on the ACT hwdge ring (ordered first: needed first);
            # x / outputs on the SP ring.
            wqkf_sb = [
                wqk_pool.tile([128, C], BF, name=f"wqkf_sb{ft}", tag=f"wqk{ft}")
                for ft in range(2 * hpc)
            ]
            for ft in range(2 * hpc):
                nc.scalar.dma_start(
                    out=wqkf_sb[ft], in_=wqk[ft * 128 : (ft + 1) * 128, :]
                )
            wv_sb = [
                wv_pool.tile([128, F], BF, name=f"wv_sb{cc}", tag=f"wv{cc}")
                for cc in range(cfg.c_tiles)
            ]
            for cc in range(cfg.c_tiles):
                nc.scalar.dma_start(
                    out=wv_sb[cc], in_=wv[cc * 128 : (cc + 1) * 128, :]
                )
            cos_sb = consts.tile([D, T], BF, name="cos_sb")
            nc.scalar.dma_start(out=cos_sb, in_=cosT[:, :])
            sin_sb = consts.tile([D, T], BF, name="sin_sb")
            nc.scalar.dma_start(out=sin_sb, in_=sinT[:, :])
            wout_sb = [
                wo_pool.tile([128, C], BF, name=f"wout_sb{h}", tag=f"wo{h}")
                for h in range(hpc)
            ]
            for h in range(hpc):
                nc.scalar.dma_start(
                    out=wout_sb[h], in_=wout[h * 128 : (h + 1) * 128, :]
                )
            tri_sb = consts.tile([128, 128], F16, name="tri_sb")
            nc.scalar.dma_start(out=tri_sb, in_=masks[:, :])
            ones_sb = consts.tile([128, 128], F16, name="ones_sb")
            nc.scalar.dma_start(out=ones_sb, in_=ones[:, :])
            ebias_sb = consts.tile([128, 1], FP, name="ebias_sb")
            nc.gpsimd.memset(ebias_sb, EXP_BIAS)

            # q/k transposed [D, T] per head (RoPE applied in place later);
            # v natural [T, F] stored as [128, t_tiles, F].
            qk_t = [
                persist.tile([D, T], BF, name=f"qk_t{ft}", tag=f"qk_t{ft}")
                for ft in range(2 * hpc)
            ]
            # v and the exp tiles are fp16 (not bf16): uniform 2-byte dtype
            # across DVE operands enables the DVE 2x/4x fast modes
            v_sb = persist.tile([128, cfg.t_tiles, F], F16, name="v_sb")

            def rope_inplace(ft, rope_tmp):
                t_cos = rope_tmp.tile([D, T], BF, tag="t_cos")
                nc.vector.tensor_mul(t_cos, qk_t[ft], cos_sb)
                t_shift = rope_tmp.tile([D, T], BF, tag="t_shift")
                nc.vector.tensor_copy(t_shift[0:64, :], qk_t[ft][64:128, :])
                nc.vector.tensor_copy(t_shift[64:128, :], qk_t[ft][0:64, :])
                nc.vector.tensor_mul(t_shift, t_shift, sin_sb)
                nc.vector.tensor_add(qk_t[ft], t_cos, t_shift)

            # ---------------- Phase A: QKV projection + RoPE ----------------
            with (
                tc.tile_pool(name="xpool", bufs=3) as xpool,
                tc.tile_pool(name="rope_tmp", bufs=2) as rope_tmp,
            ):

                def load_x_chunk(tci):
                    x_ch = []
                    for cc in range(cfg.c_tiles):
                        x_t = xpool.tile([128, 512], BF, tag=f"x{cc}")
                        nc.sync.dma_start(
                            out=x_t,
                            in_=xT[cc, :, tci * 512 : (tci + 1) * 512],
                        )
                        x_ch.append(x_t)
                    return x_ch

                with tc.tile_pool(name="qkv_ps", bufs=3, space="PSUM") as qkv_ps:
                    # A1: q/k (transposed layout), RoPE once each row done
                    for tci in range(cfg.t_chunks):
                        x_ch = load_x_chunk(tci)
                        for ft in range(2 * hpc):
                            ps_qk = qkv_ps.tile([128, 512], FP, tag="ps_qk")
                            for cc in range(cfg.c_tiles):
                                nc.tensor.matmul(
                                    ps_qk,
                                    lhsT=wqkf_sb[ft][:, cc * 128 : (cc + 1) * 128],
                                    rhs=x_ch[cc],
                                    start=(cc == 0),
                                    stop=(cc == cfg.c_tiles - 1),
                                )
                            nc.vector.tensor_copy(
                                qk_t[ft][:, tci * 512 : (tci + 1) * 512], ps_qk
                            )
                            if tci == cfg.t_chunks - 1:
                                rope_inplace(ft, rope_tmp)

                    # A2: v (natural); x re-streamed; ropes overlap on DVE
                    for tci in range(cfg.t_chunks):
                        x_ch = load_x_chunk(tci)
                        for tt in range(4):
                            ps_v = qkv_ps.tile([128, F], FP, tag="ps_v")
                            for cc in range(cfg.c_tiles):
                                nc.tensor.matmul(
                                    ps_v,
                                    lhsT=x_ch[cc][:, tt * 128 : (tt + 1) * 128],
                                    rhs=wv_sb[cc],
                                    start=(cc == 0),
                                    stop=(cc == cfg.c_tiles - 1),
                                )
                            nc.scalar.copy(v_sb[:, tci * 4 + tt, :], ps_v)

            # ---------- Phase B+D: causal attention w/ interleaved out-proj ----------
            # scoresT blocks [k, q] so attn@v needs no transposes; exp runs
            # 1024-wide over PAIRS of 128-key tiles; DVE accumulates fp16
            # partial row-sums; one fp16 ones-matmul broadcasts the total.
            otn = [[None] * cfg.t_chunks for _ in range(hpc)]
            with (
                tc.tile_pool(name="expp", bufs=4) as expp,
                tc.tile_pool(name="accp", bufs=2) as accp,
                tc.tile_pool(name="rsp", bufs=2) as rsp,
                tc.tile_pool(name="osb_pool", bufs=2) as osb_pool,
                tc.tile_pool(name="sc_ps", bufs=2, space="PSUM") as sc_ps,
                tc.tile_pool(name="av_ps", bufs=2, space="PSUM") as av_ps,
                tc.tile_pool(name="mx_ps", bufs=2, space="PSUM") as mx_ps,
            ):

                def outproj_emitters(qc):
                    """Closures each emitting one PE-chunk of out-proj for
                    query chunk qc (4 accum matmuls + Pool copy; last group
                    of each token tile also emits the DMA out)."""
                    ems = []
                    for tt_local in range(4):
                        tt = qc * 4 + tt_local
                        off = tt_local * 128
                        holder = {}
                        for n in range(cfg.n_chunks):
                            def group(holder=holder, qc=qc, off=off, n=n, tt=tt):
                                if n == 0:
                                    holder["sb"] = osb_pool.tile(
                                        [128, C], FP, name="out_sb", tag="out_sb"
                                    )
                                ps_o = mx_ps.tile(
                                    [128, 512], FP, name="ps_o", tag="mx"
                                )
                                for h in range(hpc):
                                    nc.tensor.matmul(
                                        ps_o,
                                        lhsT=otn[h][qc][:, off : off + 128],
                                        rhs=wout_sb[h][:, n * 512 : (n + 1) * 512],
                                        start=(h == 0),
                                        stop=(h == hpc - 1),
                                    )
                                # PSUM->SBUF copies split 1:3 Act:DVE so
                                # neither engine becomes the bottleneck
                                dst = holder["sb"][:, n * 512 : (n + 1) * 512]
                                if n == 0:
                                    nc.scalar.copy(dst, ps_o)
                                else:
                                    nc.vector.tensor_copy(dst, ps_o)
                                # stream the result out in halves so the final
                                # DMA tail is short
                                half = C // 2 if cfg.n_chunks >= 2 else 0
                                if cfg.n_chunks >= 2 and 2 * (n + 1) == cfg.n_chunks:
                                    nc.sync.dma_start(
                                        out=out[tt * 128 : (tt + 1) * 128, 0:half],
                                        in_=holder["sb"][:, 0:half],
                                    )
                                elif n == cfg.n_chunks - 1:
                                    nc.sync.dma_start(
                                        out=out[tt * 128 : (tt + 1) * 128, half:C],
                                        in_=holder["sb"][:, half:C],
                                    )
                            ems.append(group)
                    return ems

                pending = []  # out-proj emitters for the previous query chunk
                pending_fin = [None]  # lazy per-head softmax finisher

                def make_finisher(h, qc, ps_av, acc):
                    def fin():
                        # cross-partition rowsum total via ONE all-ones fp16
                        # matmul; 1/x = exp(-ln(x)) on ScalarE (the custom-DVE
                        # fast reciprocal doesn't compile on this walrus and
                        # the native DVE reciprocal costs 3.4us)
                        ps_rs = mx_ps.tile(
                            [128, 512], FP, name="ps_rs", tag="mx"
                        )
                        nc.tensor.matmul(
                            ps_rs, lhsT=ones_sb, rhs=acc, start=True, stop=True
                        )
                        rsln = rsp.tile([128, 512], FP, name="rsln", tag="rsln")
                        nc.scalar.activation(rsln, ps_rs, Ln)
                        rsrec = rsp.tile([128, 512], FP, name="rsrec", tag="rsrec")
                        nc.scalar.activation(rsrec, rsln, Exp, scale=-1.0)
                        o = otp.tile(
                            [128, 512], BF, name=f"otn{h}_{qc}", tag=f"otn{h}_{qc}"
                        )
                        nc.vector.tensor_mul(o, ps_av, rsrec)
                        otn[h][qc] = o
                    return fin

                for qc in range(cfg.t_chunks):
                    nkp = (qc + 1) * 2  # causal: pairs of key tiles
                    n_pairs = hpc * nkp
                    pi = 0  # pair counter within this qc
                    emitted = 0
                    # per-tile (key-tile j, q-valid-start o, width w): the 4
                    # diagonal tiles only need queries >= their own offset
                    tiles = [
                        (j, max(0, j * 128 - qc * 512))
                        for j in range(2 * nkp)
                    ]
                    for h in range(hpc):
                        q_h = qk_t[h]
                        k_h = qk_t[hpc + h]
                        q_sl = q_h[:, qc * 512 : (qc + 1) * 512]
                        ps_av = av_ps.tile([128, 512], FP, tag="ps_av")
                        acc = accp.tile([128, 512], F16, tag="acc")

                        def emit_av_adds(jp, expT, ps_av=ps_av, acc=acc, h=h):
                            pr = [tiles[2 * jp], tiles[2 * jp + 1]]
                            w0 = 512 - pr[0][1]
                            first, last = (jp == 0), (jp == nkp - 1)
                            for ti, (base, (j, o)) in enumerate(
                                ((0, pr[0]), (w0, pr[1]))
                            ):
                                nc.tensor.matmul(
                                    ps_av[:, o:512],
                                    lhsT=v_sb[:, j, h * 128 : (h + 1) * 128],
                                    rhs=expT[:, base : base + 512 - o],
                                    start=(first and ti == 0),
                                    stop=(last and ti == 1),
                                )
                            # fp16 partial row-sums on DVE (fast-mode, SBUF)
                            for ti, (base, (j, o)) in enumerate(
                                ((0, pr[0]), (w0, pr[1]))
                            ):
                                if first and ti == 0:
                                    nc.vector.tensor_copy(acc, expT[:, 0:512])
                                else:
                                    nc.vector.tensor_add(
                                        acc[:, o:512],
                                        acc[:, o:512],
                                        expT[:, base : base + 512 - o],
                                    )

                        la = []  # (jp, expT) whose attn@v emission is deferred
                        for jp in range(nkp):
                            pr = [tiles[2 * jp], tiles[2 * jp + 1]]
                            w0, w1 = (512 - pr[0][1]), (512 - pr[1][1])
                            ew = w0 + w1
                            ps_sc = sc_ps.tile([128, 1024], FP, tag="ps_sc")
                            for base, (j, o) in ((0, pr[0]), (w0, pr[1])):
                                nc.tensor.matmul(
                                    ps_sc[:, base : base + 512 - o],
                                    lhsT=k_h[:, j * 128 : (j + 1) * 128],
                                    rhs=q_sl[:, o:512],
                                    start=True,
                                    stop=True,
                                )
                            expT = expp.tile([128, 1024], F16, tag="expT")
                            nc.scalar.activation(
                                expT[:, 0:ew], ps_sc[:, 0:ew], Exp,
                                scale=float(cfg.scale), bias=ebias_sb,
                            )
                            # triangular mask on each diagonal tile's leading
                            # 128 queries
                            for base, (j, o) in ((0, pr[0]), (w0, pr[1])):
                                if o or (j * 128 == qc * 512):
                                    nc.vector.tensor_mul(
                                        expT[:, base : base + 128],
                                        expT[:, base : base + 128],
                                        tri_sb,
                                    )
                            # one-pair lookahead: defer attn@v emission so PE
                            # never catches up with ScalarE's exps
                            la.append((jp, expT))
                            if len(la) > 1:
                                emit_av_adds(*la.pop(0))
                            # finish the PREVIOUS head's softmax one pair into
                            # this head (keeps its Act/DVE tail off the
                            # critical path between heads)
                            if jp == 0 and pending_fin[0] is not None:
                                pending_fin[0]()
                                pending_fin[0] = None
                            # interleave out-proj of the previous chunk into
                            # the PE stream to fill exp-wait bubbles
                            pi += 1
                            while pending and emitted < len(pending) * pi // n_pairs:
                                pending[emitted]()
                                emitted += 1
                        for item in la:
                            emit_av_adds(*item)
                        la = []
                        pending_fin[0] = make_finisher(h, qc, ps_av, acc)
                    for em in pending[emitted:]:
                        em()
                    pending = outproj_emitters(qc)
                # flush the last head's finisher, then the last out-proj
                pending_fin[0]()
                pending_fin[0] = None
                for em in pending:
                    em()

    return nc


def rope_tables(T, dtype=np.float32):
    inv_freq = 1.0 / (ROPE_THETA ** (np.arange(0, D, 2, dtype=np.float32) / D))
    t = np.arange(T, dtype=np.float32)
    freqs = np.outer(t, inv_freq)  # [T, D/2]
    emb = np.concatenate([freqs, freqs], axis=-1)  # [T, D]
    return np.cos(emb).astype(dtype), np.sin(emb).astype(dtype)


def make_core_inputs(cfg: Cfg, x_b, w_qkv, w_out, cos, sin, hg):
    """Per-core input dict. x_b [T, C] fp32; w_qkv [C, 3C']; w_out [C', C];
    cos/sin [T, D]; hg = head-group index within the batch group."""
    T, C, hpc = cfg.T, cfg.C, cfg.hpc
    F = hpc * D
    H = w_qkv.shape[1] // 3 // D  # total heads in this (possibly shrunk) problem
    CQ = H * D

    f0 = hg * F
    xT = np.ascontiguousarray(x_b.T).astype(BF_NP).reshape(C // 128, 128, T)
    wq = w_qkv[:, f0 : f0 + F]
    wk = w_qkv[:, CQ + f0 : CQ + f0 + F]
    W = np.concatenate([wq, wk], axis=1)  # [C, 2F]
    # pack per-ft: wqk[ft*128+p, cc*128+f] = W[cc*128+p, ft*128+f]
    nft, ncc = 2 * hpc, C // 128
    wqk = np.ascontiguousarray(
        W.reshape(ncc, 128, nft, 128).transpose(2, 1, 0, 3).reshape(
            nft * 128, ncc * 128
        )
    ).astype(BF_NP)
    wv = np.ascontiguousarray(w_qkv[:, 2 * CQ + f0 : 2 * CQ + f0 + F]).astype(BF_NP)
    wout = np.ascontiguousarray(w_out[f0 : f0 + F, :]).astype(BF_NP)

    cosT = np.ascontiguousarray(cos.T).astype(BF_NP)  # [D, T]
    sinT = np.ascontiguousarray(sin.T).astype(np.float32)
    sinT[0:64, :] *= -1.0  # bake rotate_half sign
    sinT = sinT.astype(BF_NP)

    # triangular mask for diagonal 128x128 blocks: tri[k, q] = 1 iff k <= q
    k_idx = np.arange(128)[:, None]
    q_idx = np.arange(128)[None, :]
    m = (k_idx <= q_idx).astype(F16_NP)

    return {
        "xT": xT,
        "wqk": wqk,
        "wv": wv,
        "wout": wout,
        "cosT": cosT,
        "sinT": sinT,
        "masks": np.ascontiguousarray(m),
        "ones": np.ones((128, 128), dtype=F16_NP),
    }


_NC_CACHE = {}


def _get_nc(cfg: Cfg):
    key = (cfg.T, cfg.C, cfg.hpc)
    if key not in _NC_CACHE:
        nc = build_attention(cfg)
        _split_multi_waits(nc)  # HW codegen needs ≤1 wait per instruction
        _NC_CACHE[key] = nc
    return _NC_CACHE[key]


def kernel(x, cos, sin, w_qkv, w_out, trace=False, tmpdir=None):
    """Full-problem entry point: full inputs in, full [B, T, C] output back."""
    from concourse.bass_utils import run_bass_kernel_spmd

    x = np.asarray(x, dtype=np.float32)
    cos = np.asarray(cos, dtype=np.float32)
    sin = np.asarray(sin, dtype=np.float32)
    w_qkv = np.asarray(w_qkv, dtype=np.float32)
    w_out = np.asarray(w_out, dtype=np.float32)

    cfg = Cfg()
    nc = _get_nc(cfg)

    in_maps = []
    for c in range(N_CORES):
        b, hg = c // 4, c % 4
        in_maps.append(
            make_core_inputs(cfg, x[b], w_qkv, w_out, cos, sin, hg)
        )

    res = run_bass_kernel_spmd(
        nc,
        in_maps,
        core_ids=list(range(N_CORES)),
        trace=trace,
        tmpdir=tmpdir,
    )
    partials = [r["out"] for r in res.results]
    out = np.empty((B, cfg.T, cfg.C), dtype=np.float32)
    for b in range(B):
        out[b] = partials[4 * b] + partials[4 * b + 1]
        out[b] += partials[4 * b + 2]
        out[b] += partials[4 * b + 3]
    if trace:
        return out, res
    return out
